# revision 1
# baseline (speedup 1.0000x reference)
"""Trainium2 Bass kernel for the contrastive loss:

    epos = exp(cos_sim(q_pos, img_pos))   # [2B] rows, D=1024
    eneg = exp(cos_sim(q_neg, img_neg))   # [23B]
    pos_sum = segsum(epos, 2); neg_sum = segsum(eneg, 23)   # [B]
    loss = sum((neg_sum - pos_sum) / (pos_sum + neg_sum + 0.001))

Data-parallel over 8 NeuronCores: core c takes batch items [c*512, (c+1)*512),
i.e. rows [c*1024,(c+1)*1024) of the pos tensors and [c*11776,(c+1)*11776) of
the neg tensors. Each core emits its 512 per-item values; the host sums.

Per-core layout: local item i = 4*p + s (partition p in [0,128), slot s in
[0,4)), so partition p owns pos rows 8p..8p+7 and neg rows 92p..92p+91 of the
core's shard — each partition's rows are contiguous in DRAM, so every DMA is
128 partitions x (4 rows * 4KiB) contiguous.

Per 128-row slice [128, 1024]: the row-wise dot runs on the vector engine as
one fused scalar_tensor_tensor ((a*1)*b with accum_out), and the two
sum-of-squares run on the scalar engine as Square activations with accum_out.
A fraction of the b-squares is moved to the vector engine to balance the two
engines; both stay below the DMA floor (~100 MiB/core through 16 SDMA
engines).

cos and e=exp(cos) are computed per chunk as stats complete, using
1/sqrt(x) = exp(-0.5*ln(x)) so the scalar engine needs only the
natural_log_exp_and_others table set (square/ln/exp) for the entire kernel —
no ~2.7us ACT table switches in the final tail. The tail is just the two
segmented reductions and the per-item fixup.
"""

import numpy as np

import concourse.bass as bass
import concourse.tile as tile
from concourse import mybir
from concourse.bass_utils import run_bass_kernel_spmd

EPS_COS = 1e-8
EP = 0.001

N_CORES = 8
P = 128            # SBUF partitions
D = 1024           # embedding dim
B_FULL = 4096      # total batch items
ITEMS = B_FULL // N_CORES   # 512 items per core
SLOTS = ITEMS // P          # 4 items per partition
J_POS = SLOTS * 2           # 8 pos rows per partition
J_NEG = SLOTS * 23          # 92 neg rows per partition
G = 8                       # j-slices per DMA chunk (4 MiB per tensor)

F32 = mybir.dt.float32
ALU = mybir.AluOpType
ACTF = mybir.ActivationFunctionType


def _split_multiwait_instructions(nc):
    """The walrus build here rejects >1 sync-wait per instruction; hoist extra
    waits onto single-wait NOPs placed just before the instruction."""
    ctr = 0
    for fn in nc.m.functions:
        for bb in fn.blocks:
            insts = list(bb.instructions)
            if not any(
                i.sync_info is not None and len(i.sync_info.on_wait) > 1
                for i in insts
            ):
                continue
            new_insts = []
            for inst in insts:
                si = inst.sync_info
                if si is not None and len(si.on_wait) > 1:
                    waits = list(si.on_wait)
                    is_drain = type(inst).__name__ == "InstDrain"
                    keep = [] if is_drain else waits[-1:]
                    move = waits if is_drain else waits[:-1]
                    for w in move:
                        ctr += 1
                        new_insts.append(
                            mybir.InstNoOp(
                                name=f"I-wsplit-{ctr}",
                                engine=inst.engine,
                                sync_info=mybir.SyncInfo(on_wait=[w], on_update=[]),
                                text_hint="wsplit",
                            )
                        )
                    si.on_wait = keep
                new_insts.append(inst)
            bb.instructions = new_insts


def build_bass():
    nc = bass.Bass()
    qp = nc.declare_dram_parameter("qp", [P * J_POS, D], F32, isOutput=False)
    pi = nc.declare_dram_parameter("pi", [P * J_POS, D], F32, isOutput=False)
    qn = nc.declare_dram_parameter("qn", [P * J_NEG, D], F32, isOutput=False)
    ni = nc.declare_dram_parameter("ni", [P * J_NEG, D], F32, isOutput=False)
    out = nc.declare_dram_parameter("out", [P, SLOTS], F32, isOutput=True)

    qp_v = qp[:].rearrange("(p j) d -> p j d", j=J_POS)
    pi_v = pi[:].rearrange("(p j) d -> p j d", j=J_POS)
    qn_v = qn[:].rearrange("(p j) d -> p j d", j=J_NEG)
    ni_v = ni[:].rearrange("(p j) d -> p j d", j=J_NEG)

    with tile.TileContext(nc) as tc:
        with (
            tc.tile_pool(name="io", bufs=2) as io,
            tc.tile_pool(name="st", bufs=1) as st,
        ):
            J_ALL = J_POS + J_NEG   # pos stats in cols [0,8), neg in [8,100)
            dot_all = st.tile([P, J_ALL], F32)
            na2_all = st.tile([P, J_ALL], F32)
            nb2_all = st.tile([P, J_ALL], F32)
            e_all = st.tile([P, J_ALL], F32)
            scr_v = st.tile([P, D], F32)
            scr_s = st.tile([P, D], F32)

            # Chunk schedule: the last chunks shrink (...,4,2,1,1) so the
            # serial compute after the final input load is minimal.
            def chunk_sizes(total, shrink_tail):
                if not shrink_tail:
                    assert total % G == 0
                    return [G] * (total // G)
                rem = total - 4
                assert rem % G == 0
                return [G] * (rem // G) + [2, 1, 1]

            chunks = []   # (a_view, b_view, col0, j0, gsz)
            for view_a, view_b, col0, total, shrink in (
                (qp_v, pi_v, 0, J_POS, False),
                (qn_v, ni_v, J_POS, J_NEG, True),
            ):
                j0 = 0
                for gsz in chunk_sizes(total, shrink):
                    chunks.append((view_a, view_b, col0, j0, gsz))
                    j0 += gsz
                assert j0 == total

            prod = st.tile([P, J_ALL], F32)
            cosv = st.tile([P, J_ALL], F32)

            # e[:, lo:hi] = exp(dot * exp(-0.5*ln(max(na2*nb2, eps^2)))).
            # ln/exp share the square table set: no ACT table switches.
            def _emit_e(lo, hi):
                c = slice(lo, hi)
                nc.vector.tensor_tensor(
                    out=prod[:, c], in0=na2_all[:, c], in1=nb2_all[:, c],
                    op=ALU.mult,
                )
                nc.vector.tensor_scalar(
                    out=prod[:, c], in0=prod[:, c], scalar1=EPS_COS * EPS_COS,
                    scalar2=None, op0=ALU.max,
                )
                nc.scalar.activation(out=prod[:, c], in_=prod[:, c], func=ACTF.Ln)
                nc.scalar.activation(
                    out=prod[:, c], in_=prod[:, c], func=ACTF.Exp, scale=-0.5
                )
                nc.vector.tensor_tensor(
                    out=cosv[:, c], in0=dot_all[:, c], in1=prod[:, c],
                    op=ALU.mult,
                )
                nc.scalar.activation(
                    out=e_all[:, c], in_=cosv[:, c], func=ACTF.Exp
                )

            # Streaming phase: only dots + squares, no cross-engine chains.
            # 6/11 of b-squares go to the vector engine: per-slice unit cost
            # is ~1.22us on DVE vs ~1.30us on ACT (ACT pays a 185ns
            # ACTIVATION_READ_ACCUMULATOR per accumulate), and ACT also owns
            # all 100 a-squares, so this split equalizes both engines.
            slice_idx = 0
            for a_v, b_v, col0, j0, gsz in chunks:
                a_t = io.tile([P, G, D], F32, tag="a")
                b_t = io.tile([P, G, D], F32, tag="b")
                nc.sync.dma_start(out=a_t[:, :gsz, :], in_=a_v[:, j0 : j0 + gsz, :])
                nc.sync.dma_start(out=b_t[:, :gsz, :], in_=b_v[:, j0 : j0 + gsz, :])
                for g in range(gsz):
                    j = col0 + j0 + g
                    a_sl = a_t[:, g, :]
                    b_sl = b_t[:, g, :]
                    nc.vector.scalar_tensor_tensor(
                        out=scr_v[:], in0=a_sl, scalar=1.0, in1=b_sl,
                        op0=ALU.mult, op1=ALU.mult,
                        accum_out=dot_all[:, j : j + 1],
                    )
                    nc.scalar.activation(
                        out=scr_s[:], in_=a_sl, func=ACTF.Square,
                        accum_out=na2_all[:, j : j + 1],
                    )
                    if (slice_idx % 11) < 6:
                        nc.vector.scalar_tensor_tensor(
                            out=scr_v[:], in0=b_sl, scalar=1.0, in1=b_sl,
                            op0=ALU.mult, op1=ALU.mult,
                            accum_out=nb2_all[:, j : j + 1],
                        )
                    else:
                        nc.scalar.activation(
                            out=scr_s[:], in_=b_sl, func=ACTF.Square,
                            accum_out=nb2_all[:, j : j + 1],
                        )
                    slice_idx += 1

                # Once the first 96 columns' stats are complete, compute
                # their e-values while the last chunks still stream in; the
                # final tail then only covers the last 4 columns.
                if col0 + j0 + gsz == 96:
                    _emit_e(0, 96)

            _emit_e(96, J_ALL)

            pos_sum = st.tile([P, SLOTS], F32)
            neg_sum = st.tile([P, SLOTS], F32)
            nc.vector.tensor_reduce(
                out=pos_sum[:],
                in_=e_all[:, :J_POS].rearrange("p (s t) -> p s t", t=2),
                axis=mybir.AxisListType.X,
                op=ALU.add,
            )
            nc.vector.tensor_reduce(
                out=neg_sum[:],
                in_=e_all[:, J_POS:].rearrange("p (s t) -> p s t", t=23),
                axis=mybir.AxisListType.X,
                op=ALU.add,
            )
            num = st.tile([P, SLOTS], F32)
            den = st.tile([P, SLOTS], F32)
            nc.vector.tensor_tensor(
                out=num[:], in0=neg_sum[:], in1=pos_sum[:], op=ALU.subtract
            )
            nc.vector.scalar_tensor_tensor(
                out=den[:], in0=pos_sum[:], scalar=EP, in1=neg_sum[:],
                op0=ALU.add, op1=ALU.add,
            )
            rden = st.tile([P, SLOTS], F32)
            nc.vector.reciprocal(out=rden[:], in_=den[:])
            per_item = st.tile([P, SLOTS], F32)
            nc.vector.tensor_tensor(
                out=per_item[:], in0=num[:], in1=rden[:], op=ALU.mult
            )
            nc.sync.dma_start(out=out[:], in_=per_item[:])

    _split_multiwait_instructions(nc)
    return nc


_NC_CACHE = None


def _get_nc():
    global _NC_CACHE
    if _NC_CACHE is None:
        _NC_CACHE = build_bass()
    return _NC_CACHE


def kernel(question_embeddings_pos, question_embeddings_neg,
           pos_image_embeddings, neg_image_embeddings, batch_size=None,
           **_unused):
    qp = np.ascontiguousarray(np.asarray(question_embeddings_pos, dtype=np.float32))
    qn = np.ascontiguousarray(np.asarray(question_embeddings_neg, dtype=np.float32))
    pi = np.ascontiguousarray(np.asarray(pos_image_embeddings, dtype=np.float32))
    ni = np.ascontiguousarray(np.asarray(neg_image_embeddings, dtype=np.float32))

    rp = 2 * ITEMS   # pos rows per core
    rn = 23 * ITEMS  # neg rows per core
    in_maps = [
        {
            "qp": qp[c * rp : (c + 1) * rp],
            "pi": pi[c * rp : (c + 1) * rp],
            "qn": qn[c * rn : (c + 1) * rn],
            "ni": ni[c * rn : (c + 1) * rn],
        }
        for c in range(N_CORES)
    ]
    res = run_bass_kernel_spmd(_get_nc(), in_maps, list(range(N_CORES)))
    total = np.float64(0.0)
    for c in range(N_CORES):
        total += res.results[c]["out"].sum(dtype=np.float64)
    return np.float32(total)



# revision 8
# speedup vs baseline: 2.2550x; 2.2550x over previous
"""Trainium2 Bass kernel for the contrastive loss:

    epos = exp(cos_sim(q_pos, img_pos))   # [2B] rows, D=1024
    eneg = exp(cos_sim(q_neg, img_neg))   # [23B]
    pos_sum = segsum(epos, 2); neg_sum = segsum(eneg, 23)   # [B]
    loss = sum((neg_sum - pos_sum) / (pos_sum + neg_sum + 0.001))

Data-parallel over 8 NeuronCores: core c takes batch items [c*512, (c+1)*512),
i.e. rows [c*1024,(c+1)*1024) of the pos tensors and [c*11776,(c+1)*11776) of
the neg tensors. Each core emits its 512 per-item values; the host sums.

Per-core layout: local item i = 4*p + s (partition p in [0,128), slot s in
[0,4)), so partition p owns pos rows 8p..8p+7 and neg rows 92p..92p+91 of the
core's shard — each partition's rows are contiguous in DRAM, so every DMA is
128 partitions x (4 rows * 4KiB) contiguous.

Per 128-row slice [128, 1024]: the row-wise dot runs on the vector engine as
one fused scalar_tensor_tensor ((a*1)*b with accum_out), and the two
sum-of-squares run on the scalar engine as Square activations with accum_out.
A fraction of the b-squares is moved to the vector engine to balance the two
engines; both stay below the DMA floor (~100 MiB/core through 16 SDMA
engines).

cos and e=exp(cos) are computed per chunk as stats complete, using
1/sqrt(x) = exp(-0.5*ln(x)) so the scalar engine needs only the
natural_log_exp_and_others table set (square/ln/exp) for the entire kernel —
no ~2.7us ACT table switches in the final tail. The tail is just the two
segmented reductions and the per-item fixup.
"""

import ml_dtypes
import numpy as np

import concourse.bass as bass
import concourse.tile as tile
from concourse import mybir
from concourse.bass_utils import run_bass_kernel_spmd

EPS_COS = 1e-8
EP = 0.001

N_CORES = 8
P = 128            # SBUF partitions
D = 1024           # embedding dim
B_FULL = 4096      # total batch items
ITEMS = B_FULL // N_CORES   # 512 items per core
SLOTS = ITEMS // P          # 4 items per partition
J_POS = SLOTS * 2           # 8 pos rows per partition
J_NEG = SLOTS * 23          # 92 neg rows per partition
G = 8                       # j-slices per DMA chunk (4 MiB per tensor)

F32 = mybir.dt.float32
BF16 = mybir.dt.bfloat16
ALU = mybir.AluOpType
ACTF = mybir.ActivationFunctionType


def _split_multiwait_instructions(nc):
    """The walrus build here rejects >1 sync-wait per instruction; hoist extra
    waits onto single-wait NOPs placed just before the instruction."""
    ctr = 0
    for fn in nc.m.functions:
        for bb in fn.blocks:
            insts = list(bb.instructions)
            if not any(
                i.sync_info is not None and len(i.sync_info.on_wait) > 1
                for i in insts
            ):
                continue
            new_insts = []
            for inst in insts:
                si = inst.sync_info
                if si is not None and len(si.on_wait) > 1:
                    waits = list(si.on_wait)
                    is_drain = type(inst).__name__ == "InstDrain"
                    keep = [] if is_drain else waits[-1:]
                    move = waits if is_drain else waits[:-1]
                    for w in move:
                        ctr += 1
                        new_insts.append(
                            mybir.InstNoOp(
                                name=f"I-wsplit-{ctr}",
                                engine=inst.engine,
                                sync_info=mybir.SyncInfo(on_wait=[w], on_update=[]),
                                text_hint="wsplit",
                            )
                        )
                    si.on_wait = keep
                new_insts.append(inst)
            bb.instructions = new_insts


def build_bass():
    nc = bass.Bass()
    qp = nc.declare_dram_parameter("qp", [P * J_POS, D], BF16, isOutput=False)
    pi = nc.declare_dram_parameter("pi", [P * J_POS, D], BF16, isOutput=False)
    qn = nc.declare_dram_parameter("qn", [P * J_NEG, D], BF16, isOutput=False)
    ni = nc.declare_dram_parameter("ni", [P * J_NEG, D], BF16, isOutput=False)
    out = nc.declare_dram_parameter("out", [P, SLOTS], F32, isOutput=True)

    qp_v = qp[:].rearrange("(p j) d -> p j d", j=J_POS)
    pi_v = pi[:].rearrange("(p j) d -> p j d", j=J_POS)
    qn_v = qn[:].rearrange("(p j) d -> p j d", j=J_NEG)
    ni_v = ni[:].rearrange("(p j) d -> p j d", j=J_NEG)

    with tile.TileContext(nc) as tc:
        with (
            tc.tile_pool(name="io", bufs=2) as io,
            tc.tile_pool(name="st", bufs=1) as st,
        ):
            J_ALL = J_POS + J_NEG   # pos stats in cols [0,8), neg in [8,100)
            dot_all = st.tile([P, J_ALL], F32)
            na2_all = st.tile([P, J_ALL], F32)
            nb2_all = st.tile([P, J_ALL], F32)
            e_all = st.tile([P, J_ALL], F32)
            scr_v = st.tile([P, D], BF16)
            scr_s = st.tile([P, D], BF16)

            # Chunk schedule: the last chunks shrink (...,4,2,1,1) so the
            # serial compute after the final input load is minimal.
            def chunk_sizes(total, shrink_tail):
                if not shrink_tail:
                    assert total % G == 0
                    return [G] * (total // G)
                rem = total - 4
                assert rem % G == 0
                return [G] * (rem // G) + [2, 1, 1]

            chunks = []   # (a_view, b_view, col0, j0, gsz)
            for view_a, view_b, col0, total, shrink in (
                (qp_v, pi_v, 0, J_POS, False),
                (qn_v, ni_v, J_POS, J_NEG, True),
            ):
                j0 = 0
                for gsz in chunk_sizes(total, shrink):
                    chunks.append((view_a, view_b, col0, j0, gsz))
                    j0 += gsz
                assert j0 == total

            prod = st.tile([P, J_ALL], F32)
            cosv = st.tile([P, J_ALL], F32)

            # e[:, lo:hi] = exp(dot * exp(-0.5*ln(max(na2*nb2, eps^2)))).
            # ln/exp share the square table set: no ACT table switches.
            def _emit_e(lo, hi):
                c = slice(lo, hi)
                nc.vector.tensor_tensor(
                    out=prod[:, c], in0=na2_all[:, c], in1=nb2_all[:, c],
                    op=ALU.mult,
                )
                nc.vector.tensor_scalar(
                    out=prod[:, c], in0=prod[:, c], scalar1=EPS_COS * EPS_COS,
                    scalar2=None, op0=ALU.max,
                )
                nc.scalar.activation(out=prod[:, c], in_=prod[:, c], func=ACTF.Ln)
                nc.scalar.activation(
                    out=prod[:, c], in_=prod[:, c], func=ACTF.Exp, scale=-0.5
                )
                nc.vector.tensor_tensor(
                    out=cosv[:, c], in0=dot_all[:, c], in1=prod[:, c],
                    op=ALU.mult,
                )
                nc.scalar.activation(
                    out=e_all[:, c], in_=cosv[:, c], func=ACTF.Exp
                )

            # Streaming phase: only dots + squares, no cross-engine chains.
            # 6/11 of b-squares go to the vector engine: per-slice unit cost
            # is ~1.22us on DVE vs ~1.30us on ACT (ACT pays a 185ns
            # ACTIVATION_READ_ACCUMULATOR per accumulate), and ACT also owns
            # all 100 a-squares, so this split equalizes both engines.
            slice_idx = 0
            for a_v, b_v, col0, j0, gsz in chunks:
                a_t = io.tile([P, G, D], BF16, tag="a")
                b_t = io.tile([P, G, D], BF16, tag="b")
                nc.sync.dma_start(out=a_t[:, :gsz, :], in_=a_v[:, j0 : j0 + gsz, :])
                nc.sync.dma_start(out=b_t[:, :gsz, :], in_=b_v[:, j0 : j0 + gsz, :])
                for g in range(gsz):
                    j = col0 + j0 + g
                    a_sl = a_t[:, g, :]
                    b_sl = b_t[:, g, :]
                    nc.vector.scalar_tensor_tensor(
                        out=scr_v[:], in0=a_sl, scalar=1.0, in1=b_sl,
                        op0=ALU.mult, op1=ALU.mult,
                        accum_out=dot_all[:, j : j + 1],
                    )
                    nc.scalar.activation(
                        out=scr_s[:], in_=a_sl, func=ACTF.Square,
                        accum_out=na2_all[:, j : j + 1],
                    )
                    if (slice_idx % 33) < 32:
                        nc.vector.scalar_tensor_tensor(
                            out=scr_v[:], in0=b_sl, scalar=1.0, in1=b_sl,
                            op0=ALU.mult, op1=ALU.mult,
                            accum_out=nb2_all[:, j : j + 1],
                        )
                    else:
                        nc.scalar.activation(
                            out=scr_s[:], in_=b_sl, func=ACTF.Square,
                            accum_out=nb2_all[:, j : j + 1],
                        )
                    slice_idx += 1

                # Once the first 96 columns' stats are complete, compute
                # their e-values while the last chunks still stream in; the
                # final tail then only covers the last 4 columns.
                if col0 + j0 + gsz == 96:
                    _emit_e(0, 96)

            _emit_e(96, J_ALL)

            pos_sum = st.tile([P, SLOTS], F32)
            neg_sum = st.tile([P, SLOTS], F32)
            nc.vector.tensor_reduce(
                out=pos_sum[:],
                in_=e_all[:, :J_POS].rearrange("p (s t) -> p s t", t=2),
                axis=mybir.AxisListType.X,
                op=ALU.add,
            )
            nc.vector.tensor_reduce(
                out=neg_sum[:],
                in_=e_all[:, J_POS:].rearrange("p (s t) -> p s t", t=23),
                axis=mybir.AxisListType.X,
                op=ALU.add,
            )
            num = st.tile([P, SLOTS], F32)
            den = st.tile([P, SLOTS], F32)
            nc.vector.tensor_tensor(
                out=num[:], in0=neg_sum[:], in1=pos_sum[:], op=ALU.subtract
            )
            nc.vector.scalar_tensor_tensor(
                out=den[:], in0=pos_sum[:], scalar=EP, in1=neg_sum[:],
                op0=ALU.add, op1=ALU.add,
            )
            rden = st.tile([P, SLOTS], F32)
            nc.vector.reciprocal(out=rden[:], in_=den[:])
            per_item = st.tile([P, SLOTS], F32)
            nc.vector.tensor_tensor(
                out=per_item[:], in0=num[:], in1=rden[:], op=ALU.mult
            )
            nc.sync.dma_start(out=out[:], in_=per_item[:])

    _split_multiwait_instructions(nc)
    return nc


_NC_CACHE = None


def _get_nc():
    global _NC_CACHE
    if _NC_CACHE is None:
        _NC_CACHE = build_bass()
    return _NC_CACHE


def prepare_in_maps(question_embeddings_pos, question_embeddings_neg,
                    pos_image_embeddings, neg_image_embeddings):
    qp = np.asarray(question_embeddings_pos, dtype=np.float32).astype(ml_dtypes.bfloat16)
    qn = np.asarray(question_embeddings_neg, dtype=np.float32).astype(ml_dtypes.bfloat16)
    pi = np.asarray(pos_image_embeddings, dtype=np.float32).astype(ml_dtypes.bfloat16)
    ni = np.asarray(neg_image_embeddings, dtype=np.float32).astype(ml_dtypes.bfloat16)

    rp = 2 * ITEMS   # pos rows per core
    rn = 23 * ITEMS  # neg rows per core
    return [
        {
            "qp": np.ascontiguousarray(qp[c * rp : (c + 1) * rp]),
            "pi": np.ascontiguousarray(pi[c * rp : (c + 1) * rp]),
            "qn": np.ascontiguousarray(qn[c * rn : (c + 1) * rn]),
            "ni": np.ascontiguousarray(ni[c * rn : (c + 1) * rn]),
        }
        for c in range(N_CORES)
    ]


def kernel(question_embeddings_pos, question_embeddings_neg,
           pos_image_embeddings, neg_image_embeddings, batch_size=None,
           **_unused):
    in_maps = prepare_in_maps(
        question_embeddings_pos, question_embeddings_neg,
        pos_image_embeddings, neg_image_embeddings,
    )
    res = run_bass_kernel_spmd(_get_nc(), in_maps, list(range(N_CORES)))
    total = np.float64(0.0)
    for c in range(N_CORES):
        total += res.results[c]["out"].sum(dtype=np.float64)
    return np.float32(total)

